# revision 3
# baseline (speedup 1.0000x reference)
"""BinaryDense forward on 8 Trainium2 NeuronCores.

Computes out = x @ (w_raw > 0) for x[4096,4096] f32, w_raw[4096,4096] f32.
(The straight-through-estimator forward is exactly the hard binary matmul.)

Sharding: 4 batch groups x 2 feature groups (one core each).
Per core: out.T[Nc=2048, Mc=1024] = (w_bin shard).T @ (x shard), K=4096.

Device kernel (per core):
  - stream x.T shard in 128-row k-chunks, split each f32 chunk into
    bf16 hi + bf16 lo tiles (cached in SBUF; hi+lo matmuls accumulated in
    PSUM give ~fp32 accuracy at bf16 PE throughput),
  - stream w_raw shard per (n-tile, k-chunk), binarize to bf16 {0,1} on DVE,
  - matmul with binarized w as the stationary operand (output transposed),
    accumulate 2*K/128 matmuls per PSUM bank, 8 banks in flight,
  - evict PSUM -> SBUF -> DRAM out.T.
"""

import numpy as np

_NCORES = 8
_G1 = 4  # batch groups
_G2 = 2  # feature groups
_B = 4096
_D = 4096
_F = 4096

_cache = {}


def _build(K, Mc, Nc, mode="split", repeat=1):
    """Build + compile the per-core Bass program.

    xt: [K, Mc] f32 (x shard, transposed), wr: [K, Nc] f32 (w_raw shard),
    outT: [Nc, Mc] f32.
    """
    import concourse.bacc as bacc
    import concourse.tile as tile
    from concourse import mybir

    dt = mybir.dt
    P = 128
    NT = 512  # n-tile (psum free dim)
    MC = 512  # m moving chunk
    KC = K // P
    NTC = Nc // NT
    MCC = Mc // MC
    NNC = NT // P  # stationary 128-slices per n-tile

    nc = bacc.Bacc(None, target_bir_lowering=False, debug=False, num_devices=_NCORES)

    xt_d = nc.dram_tensor("xt", [K, Mc], dt.float32, kind="ExternalInput")
    wr_d = nc.dram_tensor("wr", [K, Nc], dt.float32, kind="ExternalInput")
    outT_d = nc.dram_tensor("outT", [Nc, Mc], dt.float32, kind="ExternalOutput")

    with tile.TileContext(nc) as tc:
        with (
            tc.tile_pool(name="xcache", bufs=1) as xcache,
            tc.tile_pool(name="xstage", bufs=3) as xstage,
            tc.tile_pool(name="wpool", bufs=4) as wpool,
            tc.tile_pool(name="evpool", bufs=4) as evpool,
            tc.tile_pool(name="pspool", bufs=1, space="PSUM") as pspool,
        ):
            def emit_body():
                # ---- x preprocess: split f32 -> bf16 hi + lo (or f32r) ----
                xparts = []  # per k-chunk: tuple of cached tiles
                for k in range(KC):
                    xs = xstage.tile([P, Mc], dt.float32, name="xs", tag="xs")
                    nc.sync.dma_start(xs[:], xt_d[k * P : (k + 1) * P, :])
                    if mode == "split":
                        xhi = xcache.tile(
                            [P, Mc], dt.bfloat16, name=f"xhi{k}", tag=f"xhi{k}"
                        )
                        nc.scalar.copy(xhi[:], xs[:])
                        xlo = xcache.tile(
                            [P, Mc], dt.bfloat16, name=f"xlo{k}", tag=f"xlo{k}"
                        )
                        nc.vector.tensor_sub(xlo[:], xs[:], xhi[:])
                        xparts.append((xhi, xlo))
                    elif mode == "f32r":
                        xr = xcache.tile(
                            [P, Mc], dt.float32r, name=f"xr{k}", tag=f"xr{k}"
                        )
                        nc.vector.tensor_copy(xr[:], xs[:])
                        xparts.append((xr,))
                    elif mode == "bf16":
                        xhi = xcache.tile(
                            [P, Mc], dt.bfloat16, name=f"xhi{k}", tag=f"xhi{k}"
                        )
                        nc.scalar.copy(xhi[:], xs[:])
                        xparts.append((xhi,))
                    else:
                        raise ValueError(mode)

                wdt = {"split": dt.bfloat16, "bf16": dt.bfloat16, "f32r": dt.float32r}[
                    mode
                ]

                # ---- main: per n-tile, accumulate over k into 8 psum banks ----
                for nt in range(NTC):
                    psums = {}
                    for nn in range(NNC):
                        for mc in range(MCC):
                            psums[(nn, mc)] = pspool.tile(
                                [P, MC],
                                dt.float32,
                                name=f"ps{nn}_{mc}",
                                tag=f"ps{nn}_{mc}",
                            )
                    nparts = len(xparts[0])
                    for k in range(KC):
                        wf = wpool.tile([P, NT], dt.float32, name="wf", tag="wf")
                        nc.sync.dma_start(
                            wf[:], wr_d[k * P : (k + 1) * P, nt * NT : (nt + 1) * NT]
                        )
                        wb = wpool.tile([P, NT], wdt, name="wb", tag="wb")
                        nc.vector.tensor_scalar(
                            wb[:], wf[:], 0.0, None, mybir.AluOpType.is_gt
                        )
                        for nn in range(NNC):
                            for pi in range(nparts):
                                for mc in range(MCC):
                                    nc.tensor.matmul(
                                        psums[(nn, mc)][:],
                                        wb[:, nn * P : (nn + 1) * P],
                                        xparts[k][pi][:, mc * MC : (mc + 1) * MC],
                                        start=(k == 0 and pi == 0),
                                        stop=(k == KC - 1 and pi == nparts - 1),
                                    )
                    for nn in range(NNC):
                        for mc in range(MCC):
                            ev = evpool.tile([P, MC], dt.float32, name="ev", tag="ev")
                            nc.vector.tensor_copy(ev[:], psums[(nn, mc)][:])
                            nc.sync.dma_start(
                                outT_d[
                                    nt * NT + nn * P : nt * NT + (nn + 1) * P,
                                    mc * MC : (mc + 1) * MC,
                                ],
                                ev[:],
                            )

            if repeat == 1:
                emit_body()
            else:
                with tc.For_i(0, repeat, 1):
                    emit_body()

    nc.compile()
    return nc


def _get_nc(K, Mc, Nc, mode="split", repeat=1):
    key = (K, Mc, Nc, mode, repeat)
    if key not in _cache:
        _cache[key] = _build(K, Mc, Nc, mode, repeat)
    return _cache[key]


def _run(x, w_raw, mode="split", repeat=1):
    """Shard, run on 8 cores, gather. x:[B,D] f32, w_raw:[D,F] f32."""
    from concourse.bass_utils import run_bass_kernel_spmd

    B, D = x.shape
    D2, F = w_raw.shape
    assert D == D2
    Mc = B // _G1
    Nc = F // _G2

    nc = _get_nc(D, Mc, Nc, mode, repeat)

    xt = np.ascontiguousarray(x.T)  # [D, B]
    in_maps = []
    for c in range(_NCORES):
        i, j = c // _G2, c % _G2
        in_maps.append(
            {
                "xt": np.ascontiguousarray(xt[:, i * Mc : (i + 1) * Mc]),
                "wr": np.ascontiguousarray(w_raw[:, j * Nc : (j + 1) * Nc]),
            }
        )

    res = run_bass_kernel_spmd(nc, in_maps, list(range(_NCORES)))

    outT = np.empty((F, B), dtype=np.float32)
    for c in range(_NCORES):
        i, j = c // _G2, c % _G2
        outT[j * Nc : (j + 1) * Nc, i * Mc : (i + 1) * Mc] = res.results[c]["outT"]
    return np.ascontiguousarray(outT.T)


def kernel(x, w_raw):
    x = np.asarray(x, dtype=np.float32)
    w_raw = np.asarray(w_raw, dtype=np.float32)
    return _run(x, w_raw, mode="split", repeat=1)
